# revision 6
# baseline (speedup 1.0000x reference)
"""AWQ fused dequant + GEMM, tensor-parallel over 8 Trainium2 NeuronCores.

Problem: out[b,s,n] = sum_k x[b,s,k] * W[n,k] + bias[n]
         W[n,k] = qweight[n,k] * scales[k//gs, n] + scaled_zeros[k//gs, n]
Shapes:  x [4,512,4096] fp16, qweight [11008,4096] int32 (values 0..15),
         scales/scaled_zeros [32,11008] fp16, bias [11008] fp16, gs=128.

Strategy (column-parallel, no collectives):
  - Shard N=11008 across 8 cores (1376 each); x replicated.
  - Host prep per core: pack [q (as fp16) | scales bcast | zeros bcast]
    into one [KT, 128, 3*NC] tensor ("dsp"), one DMA per k-tile, so each
    on-chip dequant DVE op has exactly one DMA dependency (the TRN2
    TensorTensor instruction encodes a single sync-wait).
  - x transposed/blocked on host to [mt][k_in][kt*128+m_in].
  - On chip per core: dequant W[k] tiles [128, NC] fp16 (resident,
    ~86KB/partition), stream x m-tiles, accumulate out[m,:] over KT
    k-matmuls per PSUM chunk (512/512/352), epilogue adds bias during
    the PSUM->SBUF copy, store [128, NC] fp16 per m-tile.
"""

import numpy as np

import concourse.bass as bass
import concourse.mybir as mybir
from concourse import bacc
from concourse.tile import TileContext
from concourse.bass_utils import run_bass_kernel_spmd

P = 128          # partitions / PE tile
N_CORES = 8
PSUM_CHUNK = 512


def _chunks(n, step=PSUM_CHUNK):
    out, c0 = [], 0
    while c0 < n:
        out.append((c0, min(step, n - c0)))
        c0 += step
    return out


def build_bass(M, K, NC):
    """Build the per-core SPMD Bass program.

    M tokens, K contraction, NC out-features per core. DRAM parameter
    layouts (host-prepped):
      xp  [MT, P, K]     fp16, xp[mt, k_in, kt*P + m_in] = x[mt*P+m_in, kt*P+k_in]
      dsp [KT, P, 3*NC]  fp16, per k-tile [ qT fp16 | scales bcast | zeros bcast ]
      bp  [P, NC]        fp16, bias broadcast across partitions
      op  [MT, P, NC]    fp16 output, op[mt, m_in, n]
    """
    MT, KT = M // P, K // P
    f16, f32 = mybir.dt.float16, mybir.dt.float32
    mult, add = mybir.AluOpType.mult, mybir.AluOpType.add

    nc = bacc.Bacc(None, target_bir_lowering=False, debug=True)
    x_in = nc.declare_dram_parameter("xp", [MT, P, K], f16, isOutput=False)
    d_in = nc.declare_dram_parameter("dsp", [KT, P, 3 * NC], f16, isOutput=False)
    b_in = nc.declare_dram_parameter("bp", [P, NC], f16, isOutput=False)
    o_out = nc.declare_dram_parameter("op", [MT, P, NC], f16, isOutput=True)

    with TileContext(nc) as tc:
        with (
            tc.tile_pool(name="wpool", bufs=KT) as wpool,
            tc.tile_pool(name="xpool", bufs=3) as xpool,
            tc.tile_pool(name="dpool", bufs=3) as dpool,
            tc.tile_pool(name="cpool", bufs=1) as cpool,
            tc.tile_pool(name="opool", bufs=MT) as opool,
            tc.tile_pool(name="pspool", bufs=6, space="PSUM") as pspool,
        ):
            bias_t = cpool.tile([P, NC], f16)
            nc.sync.dma_start(out=bias_t[:], in_=b_in[:])
            # Make DVE observe the bias DMA's semaphore lane early, so the
            # epilogue adds don't need a second sync-wait slot for it.
            scratch = cpool.tile([P, 1], f16)
            nc.vector.tensor_copy(out=scratch[:], in_=bias_t[:, 0:1])

            # Dequantize all of W for this core; tiles stay resident.
            w_tiles = [
                wpool.tile([P, NC], f16, tag="w", name=f"w{kt}") for kt in range(KT)
            ]
            for kt in range(KT):
                dt_ = dpool.tile([P, 3 * NC], f16, tag="d")
                nc.sync.dma_start(out=dt_[:], in_=d_in[kt])
                w = w_tiles[kt]
                nc.vector.tensor_tensor(
                    out=w[:], in0=dt_[:, 0:NC], in1=dt_[:, NC:2 * NC], op=mult
                )
                nc.vector.tensor_tensor(
                    out=w[:], in0=w[:], in1=dt_[:, 2 * NC:3 * NC], op=add
                )

            # Main GEMM: stream x m-tiles, W resident.
            for mt in range(MT):
                xt = xpool.tile([P, K], f16, tag="x")
                nc.sync.dma_start(out=xt[:], in_=x_in[mt])
                ot = opool.tile([P, NC], f16, tag="o")
                for (c0, csz) in _chunks(NC):
                    ps = pspool.tile([P, PSUM_CHUNK], f32, tag="ps")
                    for kt in range(KT):
                        nc.tensor.matmul(
                            ps[:, :csz],
                            xt[:, kt * P:(kt + 1) * P],
                            w_tiles[kt][:, c0:c0 + csz],
                            start=(kt == 0),
                            stop=(kt == KT - 1),
                        )
                    nc.vector.tensor_tensor(
                        out=ot[:, c0:c0 + csz], in0=ps[:, :csz],
                        in1=bias_t[:, c0:c0 + csz], op=add,
                    )
                nc.sync.dma_start(out=o_out[mt], in_=ot[:])
    nc.finalize()
    return nc


def prep_inputs(x, qweight, scales, scaled_zeros, bias):
    """Host-side shard + relayout. Returns per-core in_maps."""
    B, S, K = x.shape
    N = qweight.shape[0]
    M = B * S
    NC = N // N_CORES
    MT, KT = M // P, K // P

    # x: [M, K] -> [mt, k_in, kt, m_in], replicated to every core.
    x2 = np.ascontiguousarray(
        x.reshape(MT, P, KT, P).transpose(0, 3, 2, 1)
    ).reshape(MT, P, K)

    qT = qweight.astype(np.float16).T  # [K, N], values 0..15 exact

    in_maps = []
    for c in range(N_CORES):
        n0 = c * NC
        dsp = np.empty((KT, P, 3 * NC), np.float16)
        dsp[:, :, 0:NC] = qT[:, n0:n0 + NC].reshape(KT, P, NC)
        dsp[:, :, NC:2 * NC] = scales[:, n0:n0 + NC][:, None, :]
        dsp[:, :, 2 * NC:3 * NC] = scaled_zeros[:, n0:n0 + NC][:, None, :]
        bc = np.ascontiguousarray(np.broadcast_to(bias[n0:n0 + NC], (P, NC)))
        in_maps.append({"xp": x2, "dsp": dsp, "bp": bc})
    return in_maps


_PROG_CACHE = {}


def get_prog(M, K, NC):
    key = (M, K, NC)
    if key not in _PROG_CACHE:
        _PROG_CACHE[key] = build_bass(M, K, NC)
    return _PROG_CACHE[key]


def kernel(x, qweight, scales, scaled_zeros, bias, group_size):
    x = np.asarray(x)
    qweight = np.asarray(qweight)
    scales = np.asarray(scales)
    scaled_zeros = np.asarray(scaled_zeros)
    bias = np.asarray(bias)
    assert int(group_size) == P, f"group_size must be {P}"

    B, S, K = x.shape
    N = qweight.shape[0]
    M = B * S
    NC = N // N_CORES

    prog = get_prog(M, K, NC)
    in_maps = prep_inputs(x, qweight, scales, scaled_zeros, bias)
    res = run_bass_kernel_spmd(prog, in_maps, list(range(N_CORES))).results
    out = np.concatenate(
        [res[c]["op"].reshape(M, NC) for c in range(N_CORES)], axis=1
    )
    return out.reshape(B, S, N).astype(np.float16)


# revision 8
# speedup vs baseline: 6.8669x; 6.8669x over previous
"""AWQ fused dequant + GEMM, tensor-parallel over 8 Trainium2 NeuronCores.

Problem: out[b,s,n] = sum_k x[b,s,k] * W[n,k] + bias[n]
         W[n,k] = qweight[n,k] * scales[k//gs, n] + scaled_zeros[k//gs, n]
Shapes:  x [4,512,4096] fp16, qweight [11008,4096] int32 (values 0..15),
         scales/scaled_zeros [32,11008] fp16, bias [11008] fp16, gs=128.

Strategy (column-parallel, no collectives):
  - Shard N=11008 across 8 cores (1376 each); x replicated.
  - Host prep per core: pack [q (as fp16) | scales bcast | zeros bcast]
    into one [KT, 128, 3*NC] tensor ("dsp"), one DMA per k-tile, so each
    on-chip dequant DVE op has exactly one DMA dependency (the TRN2
    TensorTensor instruction encodes a single sync-wait).
  - x transposed/blocked on host to [mt][k_in][kt*128+m_in].
  - On chip per core: dequant W[k] tiles [128, NC] fp16 (resident,
    ~86KB/partition), stream x m-tiles, accumulate out[m,:] over KT
    k-matmuls per PSUM chunk (512/512/352), epilogue adds bias during
    the PSUM->SBUF copy, store [128, NC] fp16 per m-tile.
"""

import numpy as np

import concourse.bass as bass
import concourse.mybir as mybir
from concourse import bacc
from concourse.tile import TileContext
from concourse.bass_utils import run_bass_kernel_spmd

P = 128          # partitions / PE tile
N_CORES = 8
PSUM_CHUNK = 512


def _chunks(n, step=PSUM_CHUNK):
    out, c0 = [], 0
    while c0 < n:
        out.append((c0, min(step, n - c0)))
        c0 += step
    return out


def build_bass(M, K, NC, repeat=1):
    """Build the per-core SPMD Bass program.

    M tokens, K contraction, NC out-features per core. DRAM parameter
    layouts (host-prepped):
      xp  [MT, P, K]     fp16, xp[mt, k_in, kt*P + m_in] = x[mt*P+m_in, kt*P+k_in]
      dsp [KT, P, 3*NC]  fp16, per k-tile [ qT fp16 | scales bcast | zeros bcast ]
      bp  [P, NC]        fp16, bias broadcast across partitions
      op  [MT, P, NC]    fp16 output, op[mt, m_in, n]
    """
    MT, KT = M // P, K // P
    f16, f32 = mybir.dt.float16, mybir.dt.float32
    mult, add = mybir.AluOpType.mult, mybir.AluOpType.add

    nc = bacc.Bacc(None, target_bir_lowering=False, debug=True)
    x_in = nc.declare_dram_parameter("xp", [MT, P, K], f16, isOutput=False)
    d_in = nc.declare_dram_parameter("dsp", [KT, P, 3 * NC], f16, isOutput=False)
    b_in = nc.declare_dram_parameter("bp", [P, NC], f16, isOutput=False)
    o_out = nc.declare_dram_parameter("op", [MT, P, NC], f16, isOutput=True)

    with TileContext(nc) as tc:
        with (
            tc.tile_pool(name="wpool", bufs=KT) as wpool,
            tc.tile_pool(name="xpool", bufs=3) as xpool,
            tc.tile_pool(name="dpool", bufs=3) as dpool,
            tc.tile_pool(name="cpool", bufs=1) as cpool,
            tc.tile_pool(name="opool", bufs=MT) as opool,
            tc.tile_pool(name="pspool", bufs=6, space="PSUM") as pspool,
        ):
            bias_t = cpool.tile([P, NC], f16)
            nc.sync.dma_start(out=bias_t[:], in_=b_in[:])
            # Make DVE observe the bias DMA's semaphore lane early, so the
            # epilogue adds don't need a second sync-wait slot for it.
            scratch = cpool.tile([P, 1], f16)
            nc.vector.tensor_copy(out=scratch[:], in_=bias_t[:, 0:1])

            # Dequantize all of W for this core; tiles stay resident.
            w_tiles = [
                wpool.tile([P, NC], f16, tag="w", name=f"w{kt}") for kt in range(KT)
            ]
            for kt in range(KT):
                dt_ = dpool.tile([P, 3 * NC], f16, tag="d")
                nc.sync.dma_start(out=dt_[:], in_=d_in[kt])
                w = w_tiles[kt]
                nc.vector.tensor_tensor(
                    out=w[:], in0=dt_[:, 0:NC], in1=dt_[:, NC:2 * NC], op=mult
                )
                nc.vector.tensor_tensor(
                    out=w[:], in0=w[:], in1=dt_[:, 2 * NC:3 * NC], op=add
                )

            # Main GEMM: stream x m-tiles, W resident. repeat>1 reruns the
            # whole m-loop (same output) for slope-based timing only.
            for _rep in range(repeat):
                for mt in range(MT):
                    xt = xpool.tile([P, K], f16, tag="x")
                    nc.sync.dma_start(out=xt[:], in_=x_in[mt])
                    ot = opool.tile([P, NC], f16, tag="o")
                    for (c0, csz) in _chunks(NC):
                        ps = pspool.tile([P, PSUM_CHUNK], f32, tag="ps")
                        for kt in range(KT):
                            nc.tensor.matmul(
                                ps[:, :csz],
                                xt[:, kt * P:(kt + 1) * P],
                                w_tiles[kt][:, c0:c0 + csz],
                                start=(kt == 0),
                                stop=(kt == KT - 1),
                            )
                        nc.vector.tensor_tensor(
                            out=ot[:, c0:c0 + csz], in0=ps[:, :csz],
                            in1=bias_t[:, c0:c0 + csz], op=add,
                        )
                    nc.sync.dma_start(out=o_out[mt], in_=ot[:])
    nc.finalize()
    return nc


def prep_inputs(x, qweight, scales, scaled_zeros, bias):
    """Host-side shard + relayout. Returns per-core in_maps."""
    B, S, K = x.shape
    N = qweight.shape[0]
    M = B * S
    NC = N // N_CORES
    MT, KT = M // P, K // P

    # x: [M, K] -> [mt, k_in, kt, m_in], replicated to every core.
    x2 = np.ascontiguousarray(
        x.reshape(MT, P, KT, P).transpose(0, 3, 2, 1)
    ).reshape(MT, P, K)

    qT = qweight.astype(np.float16).T  # [K, N], values 0..15 exact

    in_maps = []
    for c in range(N_CORES):
        n0 = c * NC
        dsp = np.empty((KT, P, 3 * NC), np.float16)
        dsp[:, :, 0:NC] = qT[:, n0:n0 + NC].reshape(KT, P, NC)
        dsp[:, :, NC:2 * NC] = scales[:, n0:n0 + NC][:, None, :]
        dsp[:, :, 2 * NC:3 * NC] = scaled_zeros[:, n0:n0 + NC][:, None, :]
        bc = np.ascontiguousarray(np.broadcast_to(bias[n0:n0 + NC], (P, NC)))
        in_maps.append({"xp": x2, "dsp": dsp, "bp": bc})
    return in_maps


_PROG_CACHE = {}


def get_prog(M, K, NC):
    key = (M, K, NC)
    if key not in _PROG_CACHE:
        _PROG_CACHE[key] = build_bass(M, K, NC)
    return _PROG_CACHE[key]


def kernel(x, qweight, scales, scaled_zeros, bias, group_size):
    x = np.asarray(x)
    qweight = np.asarray(qweight)
    scales = np.asarray(scales)
    scaled_zeros = np.asarray(scaled_zeros)
    bias = np.asarray(bias)
    assert int(group_size) == P, f"group_size must be {P}"

    B, S, K = x.shape
    N = qweight.shape[0]
    M = B * S
    NC = N // N_CORES

    prog = get_prog(M, K, NC)
    in_maps = prep_inputs(x, qweight, scales, scaled_zeros, bias)
    res = run_bass_kernel_spmd(prog, in_maps, list(range(N_CORES))).results
    out = np.concatenate(
        [res[c]["op"].reshape(M, NC) for c in range(N_CORES)], axis=1
    )
    return out.reshape(B, S, N).astype(np.float16)
